# revision 13
# baseline (speedup 1.0000x reference)
"""Trainium2 Bass kernel for DiffusioUnpool (gnn_message_passing).

Computes, for a graph with N=12288 nodes, F=128 features, E=393216 COO edges:
    x_zero    = scatter(fea via perm)                     [N, F]
    atte_zero = scatter(tiled node_atte_coffe via perm)   [N]
    A         = coo_sum(edges) + I                        (dense in the reference)
    dinv      = 1/sqrt(A.sum(axis=1))
    x         = dinv * (A @ (x_zero * dinv))
    atte      = dinv * (A @ (atte_zero * dinv))

Instead of materializing the 604MB dense A, the kernel keeps the graph sparse:
    out[r] = dinv_r * sum_{e: src_e=r} attr_e * z[dst_e]  +  dinv_r^2 * xa[r]
where z[g] = [x_zero[g] | atte_zero[g]] * dinv[g] is a device-built fp16
table in DRAM, and xa = [x_zero | atte_zero] rows owned by the core (fp32).

Sharding: rows (and their edges bucketed by src row-tile) are row-sharded
across 8 cores, 1536 rows each.  Per 128-edge slab, a dma_gather pulls the
z rows addressed by dst (512B descriptors, dst-sorted for locality), one
fused DVE op builds the attr-scaled one-hot selection matrix from the src
offsets (is_equal against an iota + mult), and a single fp16 matmul per slab
contracts edges into a [128 rows x 129] fp32 PSUM accumulator.  Row sums
come from a padded ELL layout of attr reduced on-device; the epilogue
re-scales by the row dinv and adds the identity term in fp32.
"""

import os
import sys

import numpy as np

for _p in ("/opt/trn_rl_repo", "/root/.axon_site/_ro/trn_rl_repo"):
    if os.path.isdir(_p) and _p not in sys.path:
        sys.path.append(_p)

import concourse.bacc as bacc
import concourse.bass as bass
import concourse.mybir as mybir
import concourse.tile as tile
from concourse.bass_utils import run_bass_kernel_spmd
from concourse.tile_rust import add_dep_helper

FP32 = mybir.dt.float32
FP16 = mybir.dt.float16
I32 = mybir.dt.int32
I16 = mybir.dt.int16

N = 12288          # all_node_num
F = 128            # feature dim
ZW = 256           # z table row width (fp16): [x*dinv | atte*dinv | pad]
NCORES = 8
P = 128            # partitions
RPC = N // NCORES  # rows per core = 1536
RT = RPC // P      # row tiles per core = 12
GT = N // P        # global row tiles = 96
TPW = N // P       # free-dim length for p-major row mapping = 96
ZCH = 8            # z build chunks (TPW/ZCH t-columns each)

# Stash of the last BassKernelResults (test.py reads .exec_time_ns)
LAST_RESULTS = None
# Extra kwargs test.py can inject into run_bass_kernel_spmd (e.g. trace)
RUN_KWARGS = {}


# ---------------------------------------------------------------------------
# Host-side preparation: scatter, degree/ELL layout, edge bucketing
# ---------------------------------------------------------------------------

def host_prep(fea, perm, encoder_edge_index, encoder_edge_attr, node_atte_coffe,
              all_node_num, batch_size):
    n = int(all_node_num)
    b = int(batch_size)
    assert n == N
    fea = np.asarray(fea, dtype=np.float32)
    perm = np.asarray(perm).astype(np.int64)
    eidx = np.asarray(encoder_edge_index).astype(np.int64)
    attr = np.asarray(encoder_edge_attr, dtype=np.float32)
    natte = np.asarray(node_atte_coffe, dtype=np.float32)

    n_perm, f = fea.shape
    assert f == F
    node_num = natte.shape[0] // b
    swn = n_perm // natte.shape[0]

    # unpool scatters
    x_zero = np.zeros((N, F), dtype=np.float32)
    x_zero[perm] = fea
    win = np.broadcast_to(natte.reshape(b, 1, node_num),
                          (b, swn, node_num)).reshape(-1).astype(np.float32)
    atte_zero = np.zeros((N,), dtype=np.float32)
    atte_zero[perm] = win

    src = eidx[0]
    dst = eidx[1]
    E = src.shape[0]

    # --- ELL attr layout for on-device row sums (p-major: g = p*TPW + t) ---
    deg = np.bincount(src, minlength=N)
    deg_pad = max(4, int(-(-int(deg.max()) // 8) * 8))
    o1 = np.argsort(src, kind="stable")
    ssrc = src[o1]
    row_starts = np.zeros(N, dtype=np.int64)
    row_starts[1:] = np.cumsum(deg)[:-1]
    pos1 = np.arange(E) - row_starts[ssrc]
    ell = np.zeros((N, deg_pad), dtype=np.float32)
    ell[ssrc, pos1] = attr[o1]
    ell_dev = np.ascontiguousarray(ell.reshape(P, TPW, deg_pad))
    att_dev = np.ascontiguousarray(atte_zero.reshape(P, TPW))
    x_dev = np.ascontiguousarray(x_zero.reshape(P, TPW, F))

    # --- per-row-tile edge buckets, sorted by dst within each bucket ---
    tile_of = src // P                       # 0..GT-1
    o2 = np.lexsort((dst, tile_of))
    t2 = tile_of[o2]
    d2 = dst[o2]
    w2 = attr[o2]
    sl2 = (src[o2] - t2 * P).astype(np.float32)  # src offset within row tile
    tcnt = np.bincount(t2, minlength=GT)
    k_slabs = max(1, int(-(-int(tcnt.max()) // P)))
    k_slabs = int(-(-k_slabs // 2) * 2)          # round to even for stability
    EB = k_slabs * P
    tstart = np.zeros(GT, dtype=np.int64)
    tstart[1:] = np.cumsum(tcnt)[:-1]
    pos2 = np.arange(E) - tstart[t2]

    dstb = np.zeros((GT, EB), dtype=np.int64)
    attrb = np.zeros((GT, EB), dtype=np.float32)
    srclb = np.zeros((GT, EB), dtype=np.float32)
    dstb[t2, pos2] = d2
    attrb[t2, pos2] = w2
    srclb[t2, pos2] = sl2

    # edge i = k*128 + p of bucket T -> (k, p)
    attr3 = attrb.reshape(GT, k_slabs, P)
    srcl3 = srclb.reshape(GT, k_slabs, P)

    # int16 wrapped idx layout for dma_gather: element i at [i%16, i//16],
    # replicated across the 8 groups of 16 partitions
    w16 = dstb.astype(np.int16).reshape(GT, EB // 16, 16)      # [T, s, p16]
    w16 = np.transpose(w16, (0, 2, 1))                          # [T, 16, s]
    idx16 = np.tile(w16, (1, 8, 1))                             # [T, 128, s]

    in_maps = []
    for c in range(NCORES):
        sl = slice(c * RT, (c + 1) * RT)
        meta = np.stack([np.transpose(srcl3[sl], (2, 0, 1)),
                         np.transpose(attr3[sl], (2, 0, 1))], axis=-1)
        meta = np.ascontiguousarray(meta.astype(np.float32))     # [P, RT, K, 2]
        idx16_c = np.ascontiguousarray(idx16[sl])                # [RT, 128, EB//16]
        rows0 = c * RPC
        xa = np.zeros((RT, P, F + 1), dtype=np.float32)
        xa[:, :, :F] = x_zero[rows0:rows0 + RPC].reshape(RT, P, F)
        xa[:, :, F] = atte_zero[rows0:rows0 + RPC].reshape(RT, P)
        ell_own = np.ascontiguousarray(
            ell[rows0:rows0 + RPC].reshape(RT, P, deg_pad).transpose(1, 0, 2))
        in_maps.append({
            "xdev": x_dev,
            "ell": ell_dev,
            "att": att_dev,
            "ellown": ell_own,
            "idx16": idx16_c,
            "meta": meta,
            "xa": xa,
        })
    return in_maps, k_slabs, deg_pad


# ---------------------------------------------------------------------------
# Device program
# ---------------------------------------------------------------------------

def build_program(k_slabs, deg_pad, trn_type="TRN2"):
    EB = k_slabs * P
    nc = bacc.Bacc(trn_type, target_bir_lowering=False, debug=False)

    xdev = nc.dram_tensor("xdev", [P, TPW, F], FP32, kind="ExternalInput")
    ell = nc.dram_tensor("ell", [P, TPW, deg_pad], FP32, kind="ExternalInput")
    att = nc.dram_tensor("att", [P, TPW], FP32, kind="ExternalInput")
    ellown = nc.dram_tensor("ellown", [P, RT, deg_pad], FP32, kind="ExternalInput")
    idx16 = nc.dram_tensor("idx16", [RT, P, EB // 16], I16, kind="ExternalInput")
    meta = nc.dram_tensor("meta", [P, RT, k_slabs, 2], FP32, kind="ExternalInput")
    xa = nc.dram_tensor("xa", [RT, P, F + 1], FP32, kind="ExternalInput")
    out = nc.dram_tensor("out", [RT, P, F + 1], FP32, kind="ExternalOutput")
    zt = nc.dram_tensor("zt", [N, ZW], FP16, kind="Internal")

    with tile.TileContext(nc) as tc:
        _build(tc, nc, k_slabs, deg_pad,
               xdev, ell, att, ellown, idx16, meta, xa, out, zt)
    nc.compile()
    return nc


def _build(tc, nc, k_slabs, deg_pad,
           xdev, ell, att, ellown, idx16, meta, xa, out, zt):
    import contextlib
    EB = k_slabs * P
    TC = TPW // ZCH  # t-columns per z-build chunk
    ctx = contextlib.ExitStack()
    with ctx:
        cpool = ctx.enter_context(tc.tile_pool(name="consts", bufs=1))
        xpool = ctx.enter_context(tc.tile_pool(name="xin", bufs=2))
        zpool = ctx.enter_context(tc.tile_pool(name="zb", bufs=2))
        gpool = ctx.enter_context(tc.tile_pool(name="gather", bufs=3))
        mpool = ctx.enter_context(tc.tile_pool(name="onehot", bufs=8))
        epool = ctx.enter_context(tc.tile_pool(name="epi", bufs=3))
        pspool = ctx.enter_context(tc.tile_pool(name="ps", bufs=2, space="PSUM"))

        # ---- constants / input loads ----
        iota_f = cpool.tile([P, P], FP32)
        nc.gpsimd.iota(iota_f[:], pattern=[[1, P]], base=0, channel_multiplier=0,
                       allow_small_or_imprecise_dtypes=True)

        idx16_sb = cpool.tile([P, RT, EB // 16], I16)
        nc.sync.dma_start(idx16_sb[:], idx16[:].rearrange("t p s -> p t s"))
        meta_sb = cpool.tile([P, RT, k_slabs, 2], FP32)
        nc.sync.dma_start(meta_sb[:], meta[:])

        # ---- dinv (p-major) + own-row dinv (row-tile-major) ----
        ell_sb = cpool.tile([P, TPW, deg_pad], FP32)
        nc.sync.dma_start(ell_sb[:], ell[:])
        att_sb = cpool.tile([P, TPW], FP32)
        nc.sync.dma_start(att_sb[:], att[:])
        ellown_sb = cpool.tile([P, RT, deg_pad], FP32)
        nc.sync.dma_start(ellown_sb[:], ellown[:])

        rowsum = cpool.tile([P, TPW], FP32)
        nc.vector.tensor_reduce(rowsum[:], ell_sb[:], axis=mybir.AxisListType.X,
                                op=mybir.AluOpType.add)
        sq = cpool.tile([P, TPW], FP32)
        nc.scalar.activation(sq[:], rowsum[:], mybir.ActivationFunctionType.Sqrt,
                             bias=1.0, scale=1.0)
        dinv = cpool.tile([P, TPW], FP32)
        nc.vector.reciprocal(dinv[:], sq[:])
        vv = cpool.tile([P, TPW], FP32)
        nc.vector.tensor_tensor(out=vv[:], in0=att_sb[:], in1=dinv[:],
                                op=mybir.AluOpType.mult)

        rs_own = cpool.tile([P, RT], FP32)
        nc.vector.tensor_reduce(rs_own[:], ellown_sb[:], axis=mybir.AxisListType.X,
                                op=mybir.AluOpType.add)
        sq_own = cpool.tile([P, RT], FP32)
        nc.scalar.activation(sq_own[:], rs_own[:],
                             mybir.ActivationFunctionType.Sqrt, bias=1.0, scale=1.0)
        own_d = cpool.tile([P, RT], FP32)
        nc.vector.reciprocal(own_d[:], sq_own[:])
        own_dd = cpool.tile([P, RT], FP32)
        nc.vector.tensor_tensor(out=own_dd[:], in0=own_d[:], in1=own_d[:],
                                op=mybir.AluOpType.mult)

        # ---- build z table in DRAM: z[g] = [x|atte]*dinv as fp16 ----
        zt_pm = zt[:].rearrange("(p t) c -> p t c", p=P)  # [P, TPW, ZW]
        z_writes = []
        for ch in range(ZCH):
            t0 = ch * TC
            xc = xpool.tile([P, TC, F], FP32, tag="xc")
            nc.sync.dma_start(xc[:], xdev[:, t0:t0 + TC, :])
            zc = zpool.tile([P, TC, ZW], FP16, tag="zc")
            nc.vector.memset(zc[:], 0)
            for j in range(TC):
                nc.vector.tensor_scalar(
                    out=zc[:, j, 0:F], in0=xc[:, j, :],
                    scalar1=dinv[:, t0 + j:t0 + j + 1], scalar2=None,
                    op0=mybir.AluOpType.mult)
            nc.vector.tensor_copy(
                zc[:, :, F:F + 1],
                vv[:, t0:t0 + TC].rearrange("p (t c) -> p t c", c=1))
            w = nc.sync.dma_start(zt_pm[:, t0:t0 + TC, :], zc[:])
            z_writes.append(w)

        # ---- gathers + one-hot matmuls + epilogue per row tile ----
        for rt in range(RT):
            gt = gpool.tile([P, k_slabs, ZW], FP16, name=f"g{rt}", tag="g")
            gi = nc.gpsimd.dma_gather(gt[:], zt[:], idx16_sb[:, rt, :], EB, EB, ZW,
                                      single_packet=False)
            for w in z_writes:
                add_dep_helper(gi.ins, w.ins, sync=True,
                               reason="z table RAW before gather")

            ps = pspool.tile([P, F + 1], FP32, tag="ps")
            for k in range(k_slabs):
                m1 = mpool.tile([P, P], FP16, tag="m1")
                nc.vector.tensor_scalar(
                    out=m1[:], in0=iota_f[:],
                    scalar1=meta_sb[:, rt, k, 0:1],
                    scalar2=meta_sb[:, rt, k, 1:2],
                    op0=mybir.AluOpType.is_equal, op1=mybir.AluOpType.mult)
                nc.tensor.matmul(ps[:], lhsT=m1[:], rhs=gt[:, k, 0:F + 1],
                                 start=(k == 0), stop=(k == k_slabs - 1))

            # out = dinv_r * psum + dinv_r^2 * xa   (all fp32)
            xa_t = epool.tile([P, F + 1], FP32, tag="xa")
            nc.sync.dma_start(xa_t[:], xa[rt, :, :])
            o1 = epool.tile([P, F + 1], FP32, tag="o1")
            nc.vector.tensor_scalar(out=o1[:], in0=ps[:],
                                    scalar1=own_d[:, rt:rt + 1],
                                    scalar2=None, op0=mybir.AluOpType.mult)
            o2 = epool.tile([P, F + 1], FP32, tag="o2")
            nc.vector.tensor_scalar(out=o2[:], in0=xa_t[:],
                                    scalar1=own_dd[:, rt:rt + 1],
                                    scalar2=None, op0=mybir.AluOpType.mult)
            res = epool.tile([P, F + 1], FP32, tag="res")
            nc.vector.tensor_tensor(out=res[:], in0=o1[:], in1=o2[:],
                                    op=mybir.AluOpType.add)
            nc.sync.dma_start(out[rt, :, :], res[:])


# ---------------------------------------------------------------------------
# Entry point
# ---------------------------------------------------------------------------

def kernel(fea, perm, encoder_edge_index, encoder_edge_attr, node_atte_coffe,
           all_node_num, batch_size):
    global LAST_RESULTS
    in_maps, k_slabs, deg_pad = host_prep(
        fea, perm, encoder_edge_index, encoder_edge_attr, node_atte_coffe,
        all_node_num, batch_size)
    nc = build_program(k_slabs, deg_pad)
    res = run_bass_kernel_spmd(nc, in_maps, core_ids=list(range(NCORES)),
                               **RUN_KWARGS)
    LAST_RESULTS = res
    x = np.zeros((N, F), dtype=np.float32)
    atte = np.zeros((N,), dtype=np.float32)
    for c in range(NCORES):
        o = res.results[c]["out"].reshape(RPC, F + 1)
        x[c * RPC:(c + 1) * RPC] = o[:, :F]
        atte[c * RPC:(c + 1) * RPC] = o[:, F]
    return x, atte


# revision 14
# speedup vs baseline: 3.0641x; 3.0641x over previous
"""Trainium2 Bass kernel for DiffusioUnpool (gnn_message_passing).

Computes, for a graph with N=12288 nodes, F=128 features, E=393216 COO edges:
    x_zero    = scatter(fea via perm)                     [N, F]
    atte_zero = scatter(tiled node_atte_coffe via perm)   [N]
    A         = coo_sum(edges) + I                        (dense adjacency)
    dinv      = 1/sqrt(A.sum(axis=1))
    x         = dinv * (A @ (x_zero * dinv))
    atte      = dinv * (A @ (atte_zero * dinv))

Strategy (row-sharded across 8 cores, 1536 rows each):
    out[r] = dinv_r * sum_dst A[r,dst] * z[dst]  +  dinv_r^2 * xa[r]
with z = [x_zero | atte_zero] * dinv built on-device in fp16 and kept
SBUF-resident (3.2MB), and the dense A^T streamed from DRAM in fp16 as the
matmul stationary operand: per output row-tile, one contiguous 3.1MB DMA
brings all 96 [128x128] A^T K-tiles, and the PE accumulates 96 fp16 matmuls
(N=129) into a fp32 PSUM bank.  Row sums (degree+1) are reduced on-device
from a padded ELL layout of attr; the epilogue rescales by the row dinv and
adds the identity term in fp32.  This keeps the kernel entirely on the
HWDGE DMA path + PE: no software-DGE gathers (the Q7 descriptor-generation
rate, ~8ns/descriptor, made per-edge gathering the bottleneck).
"""

import os
import sys

import numpy as np

for _p in ("/opt/trn_rl_repo", "/root/.axon_site/_ro/trn_rl_repo"):
    if os.path.isdir(_p) and _p not in sys.path:
        sys.path.append(_p)

import concourse.bacc as bacc
import concourse.bass as bass
import concourse.mybir as mybir
import concourse.tile as tile
from concourse.bass_utils import run_bass_kernel_spmd

FP32 = mybir.dt.float32
FP16 = mybir.dt.float16

N = 12288          # all_node_num
F = 128            # feature dim
FA = F + 1         # features + atte channel
NCORES = 8
P = 128            # partitions
RPC = N // NCORES  # rows per core = 1536
RT = RPC // P      # row tiles per core = 12
GT = N // P        # node tiles = 96

# Stash of the last BassKernelResults (test.py reads .exec_time_ns)
LAST_RESULTS = None
# Extra kwargs test.py can inject into run_bass_kernel_spmd (e.g. trace)
RUN_KWARGS = {}


# ---------------------------------------------------------------------------
# Host-side preparation: scatter, dense A^T tiles, ELL degree layout
# ---------------------------------------------------------------------------

def host_prep(fea, perm, encoder_edge_index, encoder_edge_attr, node_atte_coffe,
              all_node_num, batch_size):
    n = int(all_node_num)
    b = int(batch_size)
    assert n == N
    fea = np.asarray(fea, dtype=np.float32)
    perm = np.asarray(perm).astype(np.int64)
    eidx = np.asarray(encoder_edge_index).astype(np.int64)
    attr = np.asarray(encoder_edge_attr, dtype=np.float32)
    natte = np.asarray(node_atte_coffe, dtype=np.float32)

    n_perm, f = fea.shape
    assert f == F
    node_num = natte.shape[0] // b
    swn = n_perm // natte.shape[0]

    # unpool scatters
    x_zero = np.zeros((N, F), dtype=np.float32)
    x_zero[perm] = fea
    win = np.broadcast_to(natte.reshape(b, 1, node_num),
                          (b, swn, node_num)).reshape(-1).astype(np.float32)
    atte_zero = np.zeros((N,), dtype=np.float32)
    atte_zero[perm] = win

    src = eidx[0]
    dst = eidx[1]
    E = src.shape[0]

    # --- ELL attr layout for on-device row sums (t-major: g = t*P + p) ---
    deg = np.bincount(src, minlength=N)
    deg_pad = max(4, int(-(-int(deg.max()) // 8) * 8))
    o1 = np.argsort(src, kind="stable")
    ssrc = src[o1]
    row_starts = np.zeros(N, dtype=np.int64)
    row_starts[1:] = np.cumsum(deg)[:-1]
    pos1 = np.arange(E) - row_starts[ssrc]
    ell = np.zeros((N, deg_pad), dtype=np.float32)
    ell[ssrc, pos1] = attr[o1]
    ell_dev = np.ascontiguousarray(
        ell.reshape(GT, P, deg_pad).transpose(1, 0, 2))          # [P, GT, deg]

    # [x_zero | atte_zero] in t-major node-tile layout for the z build
    xat = np.zeros((P, GT, FA), dtype=np.float32)
    xat[:, :, :F] = x_zero.reshape(GT, P, F).transpose(1, 0, 2)
    xat[:, :, F] = atte_zero.reshape(GT, P).T

    # --- dense A^T, fp16, tiled per core: at[rt, p_dst, kt, f_src] ---
    A32 = np.zeros((N, N), dtype=np.float32)
    np.add.at(A32, (src, dst), attr)
    A16 = A32.astype(np.float16)
    del A32
    # [rt_g, f_src, kt, p_dst] view of A[row, dst]
    A4 = A16.reshape(GT, P, GT, P)

    in_maps = []
    for c in range(NCORES):
        rows0 = c * RPC
        # at[rt, p_dst, kt, f_src]; partition-major contiguous per rt
        at = np.ascontiguousarray(
            A4[c * RT:(c + 1) * RT].transpose(0, 3, 2, 1))       # [RT,P,GT,P]
        xa = np.zeros((RT, P, FA), dtype=np.float32)
        xa[:, :, :F] = x_zero[rows0:rows0 + RPC].reshape(RT, P, F)
        xa[:, :, F] = atte_zero[rows0:rows0 + RPC].reshape(RT, P)
        ell_own = np.ascontiguousarray(
            ell[rows0:rows0 + RPC].reshape(RT, P, deg_pad).transpose(1, 0, 2))
        in_maps.append({
            "at": at,
            "xat": xat,
            "ell": ell_dev,
            "ellown": ell_own,
            "xa": xa,
        })
    return in_maps, deg_pad


# ---------------------------------------------------------------------------
# Device program
# ---------------------------------------------------------------------------

def build_program(deg_pad, trn_type="TRN2"):
    nc = bacc.Bacc(trn_type, target_bir_lowering=False, debug=False)

    at = nc.dram_tensor("at", [RT, P, GT, P], FP16, kind="ExternalInput")
    xat = nc.dram_tensor("xat", [P, GT, FA], FP32, kind="ExternalInput")
    ell = nc.dram_tensor("ell", [P, GT, deg_pad], FP32, kind="ExternalInput")
    ellown = nc.dram_tensor("ellown", [P, RT, deg_pad], FP32, kind="ExternalInput")
    xa = nc.dram_tensor("xa", [RT, P, FA], FP32, kind="ExternalInput")
    out = nc.dram_tensor("out", [RT, P, FA], FP32, kind="ExternalOutput")

    with tile.TileContext(nc) as tc:
        _build(tc, nc, deg_pad, at, xat, ell, ellown, xa, out)
    nc.compile()
    return nc


def _build(tc, nc, deg_pad, at, xat, ell, ellown, xa, out):
    import contextlib
    XCH = 8                 # xat load chunks
    TC = GT // XCH          # t-columns per chunk
    ctx = contextlib.ExitStack()
    with ctx:
        cpool = ctx.enter_context(tc.tile_pool(name="consts", bufs=1))
        xpool = ctx.enter_context(tc.tile_pool(name="xin", bufs=2))
        apool = ctx.enter_context(tc.tile_pool(name="atiles", bufs=3))
        epool = ctx.enter_context(tc.tile_pool(name="epi", bufs=3))
        pspool = ctx.enter_context(tc.tile_pool(name="ps", bufs=2, space="PSUM"))

        # ---- A^T streaming starts immediately (gated on nothing) ----
        at_tiles = []
        for rt in range(RT):
            att = apool.tile([P, GT, P], FP16, name=f"at{rt}", tag="at")
            nc.sync.dma_start(att[:], at[rt, :, :, :])
            at_tiles.append(att)

        # ---- dinv (t-major) + own-row dinv (row-tile-major) ----
        ell_sb = cpool.tile([P, GT, deg_pad], FP32)
        nc.sync.dma_start(ell_sb[:], ell[:])
        ellown_sb = cpool.tile([P, RT, deg_pad], FP32)
        nc.sync.dma_start(ellown_sb[:], ellown[:])

        rowsum = cpool.tile([P, GT], FP32)
        nc.vector.tensor_reduce(rowsum[:], ell_sb[:], axis=mybir.AxisListType.X,
                                op=mybir.AluOpType.add)
        sq = cpool.tile([P, GT], FP32)
        nc.scalar.activation(sq[:], rowsum[:], mybir.ActivationFunctionType.Sqrt,
                             bias=1.0, scale=1.0)
        dinv = cpool.tile([P, GT], FP32)
        nc.vector.reciprocal(dinv[:], sq[:])

        rs_own = cpool.tile([P, RT], FP32)
        nc.vector.tensor_reduce(rs_own[:], ellown_sb[:], axis=mybir.AxisListType.X,
                                op=mybir.AluOpType.add)
        sq_own = cpool.tile([P, RT], FP32)
        nc.scalar.activation(sq_own[:], rs_own[:],
                             mybir.ActivationFunctionType.Sqrt, bias=1.0, scale=1.0)
        own_d = cpool.tile([P, RT], FP32)
        nc.vector.reciprocal(own_d[:], sq_own[:])
        own_dd = cpool.tile([P, RT], FP32)
        nc.vector.tensor_tensor(out=own_dd[:], in0=own_d[:], in1=own_d[:],
                                op=mybir.AluOpType.mult)

        # ---- SBUF-resident z = [x|atte]*dinv, fp16 [P, GT, FA] ----
        z_sb = cpool.tile([P, GT, FA], FP16)
        for ch in range(XCH):
            t0 = ch * TC
            xc = xpool.tile([P, TC, FA], FP32, tag="xc")
            nc.sync.dma_start(xc[:], xat[:, t0:t0 + TC, :])
            for j in range(TC):
                nc.vector.tensor_scalar(
                    out=z_sb[:, t0 + j, :], in0=xc[:, j, :],
                    scalar1=dinv[:, t0 + j:t0 + j + 1], scalar2=None,
                    op0=mybir.AluOpType.mult)

        # ---- per row tile: 96-deep PSUM accumulation + epilogue ----
        for rt in range(RT):
            att = at_tiles[rt]
            ps = pspool.tile([P, FA], FP32, tag="ps")
            for kt in range(GT):
                nc.tensor.matmul(ps[:], lhsT=att[:, kt, :], rhs=z_sb[:, kt, :],
                                 start=(kt == 0), stop=(kt == GT - 1))

            # out = dinv_r * psum + dinv_r^2 * xa   (all fp32)
            xa_t = epool.tile([P, FA], FP32, tag="xa")
            nc.sync.dma_start(xa_t[:], xa[rt, :, :])
            o1 = epool.tile([P, FA], FP32, tag="o1")
            nc.vector.tensor_scalar(out=o1[:], in0=ps[:],
                                    scalar1=own_d[:, rt:rt + 1],
                                    scalar2=None, op0=mybir.AluOpType.mult)
            o2 = epool.tile([P, FA], FP32, tag="o2")
            nc.vector.tensor_scalar(out=o2[:], in0=xa_t[:],
                                    scalar1=own_dd[:, rt:rt + 1],
                                    scalar2=None, op0=mybir.AluOpType.mult)
            res = epool.tile([P, FA], FP32, tag="res")
            nc.vector.tensor_tensor(out=res[:], in0=o1[:], in1=o2[:],
                                    op=mybir.AluOpType.add)
            nc.sync.dma_start(out[rt, :, :], res[:])


# ---------------------------------------------------------------------------
# Entry point
# ---------------------------------------------------------------------------

def kernel(fea, perm, encoder_edge_index, encoder_edge_attr, node_atte_coffe,
           all_node_num, batch_size):
    global LAST_RESULTS
    in_maps, deg_pad = host_prep(
        fea, perm, encoder_edge_index, encoder_edge_attr, node_atte_coffe,
        all_node_num, batch_size)
    nc = build_program(deg_pad)
    res = run_bass_kernel_spmd(nc, in_maps, core_ids=list(range(NCORES)),
                               **RUN_KWARGS)
    LAST_RESULTS = res
    x = np.zeros((N, F), dtype=np.float32)
    atte = np.zeros((N,), dtype=np.float32)
    for c in range(NCORES):
        o = res.results[c]["out"].reshape(RPC, FA)
        x[c * RPC:(c + 1) * RPC] = o[:, :F]
        atte[c * RPC:(c + 1) * RPC] = o[:, F]
    return x, atte


# revision 16
# speedup vs baseline: 3.2827x; 1.0713x over previous
"""Trainium2 Bass kernel for DiffusioUnpool (gnn_message_passing).

Computes, for a graph with N=12288 nodes, F=128 features, E=393216 COO edges:
    x_zero    = scatter(fea via perm)                     [N, F]
    atte_zero = scatter(tiled node_atte_coffe via perm)   [N]
    A         = coo_sum(edges) + I                        (dense adjacency)
    dinv      = 1/sqrt(A.sum(axis=1))
    x         = dinv * (A @ (x_zero * dinv))
    atte      = dinv * (A @ (atte_zero * dinv))

Strategy (row-sharded across 8 cores, 1536 rows each):
    out[r] = dinv_r * sum_dst A[r,dst] * z[dst]  +  dinv_r^2 * xa[r]
with z = [x_zero | atte_zero] * dinv built on-device in fp16 and kept
SBUF-resident (3.2MB), and the dense A^T streamed from DRAM in fp16 as the
matmul stationary operand: per output row-tile, one contiguous 3.1MB DMA
brings all 96 [128x128] A^T K-tiles, and the PE accumulates 96 fp16 matmuls
(N=129) into a fp32 PSUM bank.  Row sums (degree+1) are reduced on-device
from a padded ELL layout of attr; the epilogue rescales by the row dinv and
adds the identity term in fp32.  This keeps the kernel entirely on the
HWDGE DMA path + PE: no software-DGE gathers (the Q7 descriptor-generation
rate, ~8ns/descriptor, made per-edge gathering the bottleneck).
"""

import os
import sys

import numpy as np

for _p in ("/opt/trn_rl_repo", "/root/.axon_site/_ro/trn_rl_repo"):
    if os.path.isdir(_p) and _p not in sys.path:
        sys.path.append(_p)

import concourse.bacc as bacc
import concourse.bass as bass
import concourse.mybir as mybir
import concourse.tile as tile
from concourse.bass_utils import run_bass_kernel_spmd

FP32 = mybir.dt.float32
FP16 = mybir.dt.float16

N = 12288          # all_node_num
F = 128            # feature dim
FA = F + 1         # features + atte channel
NCORES = 8
P = 128            # partitions
RPC = N // NCORES  # rows per core = 1536
RT = RPC // P      # row tiles per core = 12
GT = N // P        # node tiles = 96

# Stash of the last BassKernelResults (test.py reads .exec_time_ns)
LAST_RESULTS = None
# Extra kwargs test.py can inject into run_bass_kernel_spmd (e.g. trace)
RUN_KWARGS = {}


# ---------------------------------------------------------------------------
# Host-side preparation: scatter, dense A^T tiles, ELL degree layout
# ---------------------------------------------------------------------------

def host_prep(fea, perm, encoder_edge_index, encoder_edge_attr, node_atte_coffe,
              all_node_num, batch_size):
    n = int(all_node_num)
    b = int(batch_size)
    assert n == N
    fea = np.asarray(fea, dtype=np.float32)
    perm = np.asarray(perm).astype(np.int64)
    eidx = np.asarray(encoder_edge_index).astype(np.int64)
    attr = np.asarray(encoder_edge_attr, dtype=np.float32)
    natte = np.asarray(node_atte_coffe, dtype=np.float32)

    n_perm, f = fea.shape
    assert f == F
    node_num = natte.shape[0] // b
    swn = n_perm // natte.shape[0]

    # unpool scatters
    x_zero = np.zeros((N, F), dtype=np.float32)
    x_zero[perm] = fea
    win = np.broadcast_to(natte.reshape(b, 1, node_num),
                          (b, swn, node_num)).reshape(-1).astype(np.float32)
    atte_zero = np.zeros((N,), dtype=np.float32)
    atte_zero[perm] = win

    src = eidx[0]
    dst = eidx[1]
    E = src.shape[0]

    # --- ELL attr layout for on-device row sums (t-major: g = t*P + p) ---
    deg = np.bincount(src, minlength=N)
    deg_pad = max(4, int(-(-int(deg.max()) // 8) * 8))
    o1 = np.argsort(src, kind="stable")
    ssrc = src[o1]
    row_starts = np.zeros(N, dtype=np.int64)
    row_starts[1:] = np.cumsum(deg)[:-1]
    pos1 = np.arange(E) - row_starts[ssrc]
    ell = np.zeros((N, deg_pad), dtype=np.float32)
    ell[ssrc, pos1] = attr[o1]
    ell_dev = np.ascontiguousarray(
        ell.reshape(GT, P, deg_pad).transpose(1, 0, 2))          # [P, GT, deg]

    # [x_zero | atte_zero] in t-major node-tile layout for the z build
    xat = np.zeros((P, GT, FA), dtype=np.float32)
    xat[:, :, :F] = x_zero.reshape(GT, P, F).transpose(1, 0, 2)
    xat[:, :, F] = atte_zero.reshape(GT, P).T

    # --- dense A^T, fp16, tiled per core: at[rt, p_dst, kt, f_src] ---
    A32 = np.zeros((N, N), dtype=np.float32)
    np.add.at(A32, (src, dst), attr)
    A16 = A32.astype(np.float16)
    del A32
    # [rt_g, f_src, kt, p_dst] view of A[row, dst]
    A4 = A16.reshape(GT, P, GT, P)

    in_maps = []
    for c in range(NCORES):
        rows0 = c * RPC
        # at[rt, p_dst, kt, f_src]; partition-major contiguous per rt
        at = np.ascontiguousarray(
            A4[c * RT:(c + 1) * RT].transpose(0, 3, 2, 1))       # [RT,P,GT,P]
        xa = np.zeros((RT, P, FA), dtype=np.float32)
        xa[:, :, :F] = x_zero[rows0:rows0 + RPC].reshape(RT, P, F)
        xa[:, :, F] = atte_zero[rows0:rows0 + RPC].reshape(RT, P)
        ell_own = np.ascontiguousarray(
            ell[rows0:rows0 + RPC].reshape(RT, P, deg_pad).transpose(1, 0, 2))
        in_maps.append({
            "at": at,
            "xat": xat,
            "ell": ell_dev,
            "ellown": ell_own,
            "xa": xa,
        })
    return in_maps, deg_pad


# ---------------------------------------------------------------------------
# Device program
# ---------------------------------------------------------------------------

def build_program(deg_pad, trn_type="TRN2"):
    nc = bacc.Bacc(trn_type, target_bir_lowering=False, debug=False)

    at = nc.dram_tensor("at", [RT, P, GT, P], FP16, kind="ExternalInput")
    xat = nc.dram_tensor("xat", [P, GT, FA], FP32, kind="ExternalInput")
    ell = nc.dram_tensor("ell", [P, GT, deg_pad], FP32, kind="ExternalInput")
    ellown = nc.dram_tensor("ellown", [P, RT, deg_pad], FP32, kind="ExternalInput")
    xa = nc.dram_tensor("xa", [RT, P, FA], FP32, kind="ExternalInput")
    out = nc.dram_tensor("out", [RT, P, FA], FP32, kind="ExternalOutput")

    with tile.TileContext(nc) as tc:
        _build(tc, nc, deg_pad, at, xat, ell, ellown, xa, out)
    nc.compile()
    return nc


def _build(tc, nc, deg_pad, at, xat, ell, ellown, xa, out):
    import contextlib
    XCH = 8                 # xat load chunks
    TC = GT // XCH          # t-columns per chunk
    ctx = contextlib.ExitStack()
    with ctx:
        cpool = ctx.enter_context(tc.tile_pool(name="consts", bufs=1))
        xpool = ctx.enter_context(tc.tile_pool(name="xin", bufs=2))
        apool = ctx.enter_context(tc.tile_pool(name="atiles", bufs=3))
        epool = ctx.enter_context(tc.tile_pool(name="epi", bufs=3))
        pspool = ctx.enter_context(tc.tile_pool(name="ps", bufs=2, space="PSUM"))

        # ---- A^T streaming starts immediately (gated on nothing) ----
        at_tiles = []
        for rt in range(RT):
            att = apool.tile([P, GT, P], FP16, name=f"at{rt}", tag="at")
            nc.sync.dma_start(att[:], at[rt, :, :, :])
            at_tiles.append(att)

        # ---- dinv + SBUF-resident z = [x|atte]*dinv, pipelined per chunk ----
        # Everything chunked by TC node-tiles so the first matmuls can start
        # as soon as the first z tiles exist (z is one tile per node-tile).
        z_tiles = []
        dinv_chunks = []
        for ch in range(XCH):
            t0 = ch * TC
            ec = xpool.tile([P, TC, deg_pad], FP32, tag="ec")
            nc.sync.dma_start(ec[:], ell[:, t0:t0 + TC, :])
            xc = xpool.tile([P, TC, FA], FP32, tag="xc")
            nc.sync.dma_start(xc[:], xat[:, t0:t0 + TC, :])
            rsc = cpool.tile([P, TC], FP32, name=f"rs{ch}", tag="rs", bufs=1)
            nc.vector.tensor_reduce(rsc[:], ec[:], axis=mybir.AxisListType.X,
                                    op=mybir.AluOpType.add)
            sqc = cpool.tile([P, TC], FP32, name=f"sq{ch}", tag="sqc", bufs=1)
            nc.scalar.activation(sqc[:], rsc[:],
                                 mybir.ActivationFunctionType.Sqrt,
                                 bias=1.0, scale=1.0)
            dc = cpool.tile([P, TC], FP32, name=f"dc{ch}", tag="dc", bufs=1)
            nc.vector.reciprocal(dc[:], sqc[:])
            dinv_chunks.append(dc)
            for j in range(TC):
                zt = cpool.tile([P, FA], FP16, name=f"z{t0 + j}", tag=f"z{t0 + j}")
                nc.vector.tensor_scalar(
                    out=zt[:], in0=xc[:, j, :],
                    scalar1=dc[:, j:j + 1], scalar2=None,
                    op0=mybir.AluOpType.mult)
                z_tiles.append(zt)

        # ---- own-row dinv (row-tile-major) for the epilogue ----
        ellown_sb = cpool.tile([P, RT, deg_pad], FP32)
        nc.sync.dma_start(ellown_sb[:], ellown[:])
        rs_own = cpool.tile([P, RT], FP32)
        nc.vector.tensor_reduce(rs_own[:], ellown_sb[:], axis=mybir.AxisListType.X,
                                op=mybir.AluOpType.add)
        sq_own = cpool.tile([P, RT], FP32)
        nc.scalar.activation(sq_own[:], rs_own[:],
                             mybir.ActivationFunctionType.Sqrt, bias=1.0, scale=1.0)
        own_d = cpool.tile([P, RT], FP32)
        nc.vector.reciprocal(own_d[:], sq_own[:])
        own_dd = cpool.tile([P, RT], FP32)
        nc.vector.tensor_tensor(out=own_dd[:], in0=own_d[:], in1=own_d[:],
                                op=mybir.AluOpType.mult)

        # ---- per row tile: 96-deep PSUM accumulation + epilogue ----
        for rt in range(RT):
            att = at_tiles[rt]
            ps = pspool.tile([P, FA], FP32, tag="ps")
            for kt in range(GT):
                nc.tensor.matmul(ps[:], lhsT=att[:, kt, :], rhs=z_tiles[kt][:],
                                 start=(kt == 0), stop=(kt == GT - 1))

            # out = dinv_r * psum + dinv_r^2 * xa   (all fp32)
            xa_t = epool.tile([P, FA], FP32, tag="xa")
            nc.sync.dma_start(xa_t[:], xa[rt, :, :])
            o1 = epool.tile([P, FA], FP32, tag="o1")
            nc.vector.tensor_scalar(out=o1[:], in0=ps[:],
                                    scalar1=own_d[:, rt:rt + 1],
                                    scalar2=None, op0=mybir.AluOpType.mult)
            o2 = epool.tile([P, FA], FP32, tag="o2")
            nc.vector.tensor_scalar(out=o2[:], in0=xa_t[:],
                                    scalar1=own_dd[:, rt:rt + 1],
                                    scalar2=None, op0=mybir.AluOpType.mult)
            res = epool.tile([P, FA], FP32, tag="res")
            nc.vector.tensor_tensor(out=res[:], in0=o1[:], in1=o2[:],
                                    op=mybir.AluOpType.add)
            nc.sync.dma_start(out[rt, :, :], res[:])


# ---------------------------------------------------------------------------
# Entry point
# ---------------------------------------------------------------------------

def kernel(fea, perm, encoder_edge_index, encoder_edge_attr, node_atte_coffe,
           all_node_num, batch_size):
    global LAST_RESULTS
    in_maps, deg_pad = host_prep(
        fea, perm, encoder_edge_index, encoder_edge_attr, node_atte_coffe,
        all_node_num, batch_size)
    nc = build_program(deg_pad)
    res = run_bass_kernel_spmd(nc, in_maps, core_ids=list(range(NCORES)),
                               **RUN_KWARGS)
    LAST_RESULTS = res
    x = np.zeros((N, F), dtype=np.float32)
    atte = np.zeros((N,), dtype=np.float32)
    for c in range(NCORES):
        o = res.results[c]["out"].reshape(RPC, FA)
        x[c * RPC:(c + 1) * RPC] = o[:, :F]
        atte[c * RPC:(c + 1) * RPC] = o[:, F]
    return x, atte


# revision 17
# speedup vs baseline: 3.7033x; 1.1281x over previous
"""Trainium2 Bass kernel for DiffusioUnpool (gnn_message_passing).

Computes, for a graph with N=12288 nodes, F=128 features, E=393216 COO edges:
    x_zero    = scatter(fea via perm)                     [N, F]
    atte_zero = scatter(tiled node_atte_coffe via perm)   [N]
    A         = coo_sum(edges) + I                        (dense adjacency)
    dinv      = 1/sqrt(A.sum(axis=1))
    x         = dinv * (A @ (x_zero * dinv))
    atte      = dinv * (A @ (atte_zero * dinv))

Strategy (row-sharded across 8 cores, 1536 rows each):
    out[r] = dinv_r * sum_dst A[r,dst] * z[dst]  +  dinv_r^2 * xa[r]
with z = [x_zero | atte_zero] * dinv built on-device in fp16 and kept
SBUF-resident (3.2MB), and the dense A^T streamed from DRAM in fp16 as the
matmul stationary operand: per output row-tile, one contiguous 3.1MB DMA
brings all 96 [128x128] A^T K-tiles, and the PE accumulates 96 fp16 matmuls
(N=129) into a fp32 PSUM bank.  Row sums (degree+1) are reduced on-device
from a padded ELL layout of attr; the epilogue rescales by the row dinv and
adds the identity term in fp32.  This keeps the kernel entirely on the
HWDGE DMA path + PE: no software-DGE gathers (the Q7 descriptor-generation
rate, ~8ns/descriptor, made per-edge gathering the bottleneck).
"""

import os
import sys

import numpy as np

for _p in ("/opt/trn_rl_repo", "/root/.axon_site/_ro/trn_rl_repo"):
    if os.path.isdir(_p) and _p not in sys.path:
        sys.path.append(_p)

import concourse.bacc as bacc
import concourse.bass as bass
import concourse.mybir as mybir
import concourse.tile as tile
from concourse.bass_utils import run_bass_kernel_spmd

FP32 = mybir.dt.float32
FP16 = mybir.dt.float16

N = 12288          # all_node_num
F = 128            # feature dim
FA = F + 1         # features + atte channel
NCORES = 8
P = 128            # partitions
RPC = N // NCORES  # rows per core = 1536
RT = RPC // P      # row tiles per core = 12
GT = N // P        # node tiles = 96

# Stash of the last BassKernelResults (test.py reads .exec_time_ns)
LAST_RESULTS = None
# Extra kwargs test.py can inject into run_bass_kernel_spmd (e.g. trace)
RUN_KWARGS = {}


# ---------------------------------------------------------------------------
# Host-side preparation: scatter, dense A^T tiles, ELL degree layout
# ---------------------------------------------------------------------------

def host_prep(fea, perm, encoder_edge_index, encoder_edge_attr, node_atte_coffe,
              all_node_num, batch_size):
    n = int(all_node_num)
    b = int(batch_size)
    assert n == N
    fea = np.asarray(fea, dtype=np.float32)
    perm = np.asarray(perm).astype(np.int64)
    eidx = np.asarray(encoder_edge_index).astype(np.int64)
    attr = np.asarray(encoder_edge_attr, dtype=np.float32)
    natte = np.asarray(node_atte_coffe, dtype=np.float32)

    n_perm, f = fea.shape
    assert f == F
    node_num = natte.shape[0] // b
    swn = n_perm // natte.shape[0]

    # unpool scatters
    x_zero = np.zeros((N, F), dtype=np.float32)
    x_zero[perm] = fea
    win = np.broadcast_to(natte.reshape(b, 1, node_num),
                          (b, swn, node_num)).reshape(-1).astype(np.float32)
    atte_zero = np.zeros((N,), dtype=np.float32)
    atte_zero[perm] = win

    src = eidx[0]
    dst = eidx[1]
    E = src.shape[0]

    # --- ELL attr layout for on-device row sums (t-major: g = t*P + p) ---
    deg = np.bincount(src, minlength=N)
    deg_pad = max(4, int(-(-int(deg.max()) // 8) * 8))
    o1 = np.argsort(src, kind="stable")
    ssrc = src[o1]
    row_starts = np.zeros(N, dtype=np.int64)
    row_starts[1:] = np.cumsum(deg)[:-1]
    pos1 = np.arange(E) - row_starts[ssrc]
    ell = np.zeros((N, deg_pad), dtype=np.float32)
    ell[ssrc, pos1] = attr[o1]
    ell_dev = np.ascontiguousarray(
        ell.reshape(GT, P, deg_pad).transpose(1, 0, 2)).astype(np.float16)

    # [x_zero | atte_zero] in t-major node-tile layout for the z build
    xat = np.zeros((P, GT, FA), dtype=np.float32)
    xat[:, :, :F] = x_zero.reshape(GT, P, F).transpose(1, 0, 2)
    xat[:, :, F] = atte_zero.reshape(GT, P).T

    # --- dense A^T, fp16, tiled per core: at[rt, p_dst, kt, f_src] ---
    A32 = np.zeros((N, N), dtype=np.float32)
    np.add.at(A32, (src, dst), attr)
    A16 = A32.astype(np.float16)
    del A32
    # [rt_g, f_src, kt, p_dst] view of A[row, dst]
    A4 = A16.reshape(GT, P, GT, P)

    in_maps = []
    for c in range(NCORES):
        rows0 = c * RPC
        # at[rt, p_dst, kt, f_src]; partition-major contiguous per rt
        at = np.ascontiguousarray(
            A4[c * RT:(c + 1) * RT].transpose(0, 3, 2, 1))       # [RT,P,GT,P]
        xa = np.zeros((RT, P, FA), dtype=np.float32)
        xa[:, :, :F] = x_zero[rows0:rows0 + RPC].reshape(RT, P, F)
        xa[:, :, F] = atte_zero[rows0:rows0 + RPC].reshape(RT, P)
        ell_own = np.ascontiguousarray(
            ell[rows0:rows0 + RPC].reshape(RT, P, deg_pad)
            .transpose(1, 0, 2)).astype(np.float16)
        in_maps.append({
            "at": at,
            "xat": xat.astype(np.float16),
            "ell": ell_dev,
            "ellown": ell_own,
            "xa": xa,
        })
    return in_maps, deg_pad


# ---------------------------------------------------------------------------
# Device program
# ---------------------------------------------------------------------------

def build_program(deg_pad, trn_type="TRN2"):
    nc = bacc.Bacc(trn_type, target_bir_lowering=False, debug=False)

    at = nc.dram_tensor("at", [RT, P, GT, P], FP16, kind="ExternalInput")
    xat = nc.dram_tensor("xat", [P, GT, FA], FP16, kind="ExternalInput")
    ell = nc.dram_tensor("ell", [P, GT, deg_pad], FP16, kind="ExternalInput")
    ellown = nc.dram_tensor("ellown", [P, RT, deg_pad], FP16, kind="ExternalInput")
    xa = nc.dram_tensor("xa", [RT, P, FA], FP32, kind="ExternalInput")
    out = nc.dram_tensor("out", [RT, P, FA], FP32, kind="ExternalOutput")

    with tile.TileContext(nc) as tc:
        _build(tc, nc, deg_pad, at, xat, ell, ellown, xa, out)
    nc.compile()
    return nc


def _build(tc, nc, deg_pad, at, xat, ell, ellown, xa, out):
    import contextlib
    XCH = 8                 # xat load chunks
    TC = GT // XCH          # t-columns per chunk
    ctx = contextlib.ExitStack()
    with ctx:
        cpool = ctx.enter_context(tc.tile_pool(name="consts", bufs=1))
        xpool = ctx.enter_context(tc.tile_pool(name="xin", bufs=2))
        apool = ctx.enter_context(tc.tile_pool(name="atiles", bufs=4))
        epool = ctx.enter_context(tc.tile_pool(name="epi", bufs=3))
        pspool = ctx.enter_context(tc.tile_pool(name="ps", bufs=2, space="PSUM"))

        # ---- dinv + SBUF-resident z = [x|atte]*dinv, pipelined per chunk ----
        # Everything chunked by TC node-tiles so the first matmuls can start
        # as soon as the first z tiles exist (z is one tile per node-tile).
        z_tiles = []
        dinv_chunks = []
        for ch in range(XCH):
            t0 = ch * TC
            ec = xpool.tile([P, TC, deg_pad], FP16, tag="ec")
            nc.sync.dma_start(ec[:], ell[:, t0:t0 + TC, :])
            xc = xpool.tile([P, TC, FA], FP16, tag="xc")
            nc.sync.dma_start(xc[:], xat[:, t0:t0 + TC, :])
            rsc = cpool.tile([P, TC], FP32, name=f"rs{ch}", tag="rs", bufs=1)
            nc.vector.tensor_reduce(rsc[:], ec[:], axis=mybir.AxisListType.X,
                                    op=mybir.AluOpType.add)
            sqc = cpool.tile([P, TC], FP32, name=f"sq{ch}", tag="sqc", bufs=1)
            nc.scalar.activation(sqc[:], rsc[:],
                                 mybir.ActivationFunctionType.Sqrt,
                                 bias=1.0, scale=1.0)
            dc = cpool.tile([P, TC], FP32, name=f"dc{ch}", tag="dc", bufs=1)
            nc.vector.reciprocal(dc[:], sqc[:])
            dinv_chunks.append(dc)
            for j in range(TC):
                zt = cpool.tile([P, FA], FP16, name=f"z{t0 + j}", tag=f"z{t0 + j}")
                nc.vector.tensor_scalar(
                    out=zt[:], in0=xc[:, j, :],
                    scalar1=dc[:, j:j + 1], scalar2=None,
                    op0=mybir.AluOpType.mult)
                z_tiles.append(zt)

        # ---- A^T streaming on the ACT HWDGE ring (own FIFO) ----
        at_tiles = []
        for rt in range(RT):
            att = apool.tile([P, GT, P], FP16, name=f"at{rt}", tag="at")
            nc.scalar.dma_start(att[:], at[rt, :, :, :])
            at_tiles.append(att)

        # ---- own-row dinv (row-tile-major) for the epilogue ----
        ellown_sb = cpool.tile([P, RT, deg_pad], FP16)
        nc.sync.dma_start(ellown_sb[:], ellown[:])
        rs_own = cpool.tile([P, RT], FP32)
        nc.vector.tensor_reduce(rs_own[:], ellown_sb[:], axis=mybir.AxisListType.X,
                                op=mybir.AluOpType.add)
        sq_own = cpool.tile([P, RT], FP32)
        nc.scalar.activation(sq_own[:], rs_own[:],
                             mybir.ActivationFunctionType.Sqrt, bias=1.0, scale=1.0)
        own_d = cpool.tile([P, RT], FP32)
        nc.vector.reciprocal(own_d[:], sq_own[:])
        own_dd = cpool.tile([P, RT], FP32)
        nc.vector.tensor_tensor(out=own_dd[:], in0=own_d[:], in1=own_d[:],
                                op=mybir.AluOpType.mult)

        # ---- per row tile: 96-deep PSUM accumulation + epilogue ----
        for rt in range(RT):
            att = at_tiles[rt]
            ps = pspool.tile([P, FA], FP32, tag="ps")
            for kt in range(GT):
                nc.tensor.matmul(ps[:], lhsT=att[:, kt, :], rhs=z_tiles[kt][:],
                                 start=(kt == 0), stop=(kt == GT - 1))

            # out = dinv_r * psum + dinv_r^2 * xa   (all fp32)
            xa_t = epool.tile([P, FA], FP32, tag="xa")
            nc.sync.dma_start(xa_t[:], xa[rt, :, :])
            o1 = epool.tile([P, FA], FP32, tag="o1")
            nc.vector.tensor_scalar(out=o1[:], in0=ps[:],
                                    scalar1=own_d[:, rt:rt + 1],
                                    scalar2=None, op0=mybir.AluOpType.mult)
            o2 = epool.tile([P, FA], FP32, tag="o2")
            nc.vector.tensor_scalar(out=o2[:], in0=xa_t[:],
                                    scalar1=own_dd[:, rt:rt + 1],
                                    scalar2=None, op0=mybir.AluOpType.mult)
            res = epool.tile([P, FA], FP32, tag="res")
            nc.vector.tensor_tensor(out=res[:], in0=o1[:], in1=o2[:],
                                    op=mybir.AluOpType.add)
            nc.sync.dma_start(out[rt, :, :], res[:])


# ---------------------------------------------------------------------------
# Entry point
# ---------------------------------------------------------------------------

def kernel(fea, perm, encoder_edge_index, encoder_edge_attr, node_atte_coffe,
           all_node_num, batch_size):
    global LAST_RESULTS
    in_maps, deg_pad = host_prep(
        fea, perm, encoder_edge_index, encoder_edge_attr, node_atte_coffe,
        all_node_num, batch_size)
    nc = build_program(deg_pad)
    res = run_bass_kernel_spmd(nc, in_maps, core_ids=list(range(NCORES)),
                               **RUN_KWARGS)
    LAST_RESULTS = res
    x = np.zeros((N, F), dtype=np.float32)
    atte = np.zeros((N,), dtype=np.float32)
    for c in range(NCORES):
        o = res.results[c]["out"].reshape(RPC, FA)
        x[c * RPC:(c + 1) * RPC] = o[:, :F]
        atte[c * RPC:(c + 1) * RPC] = o[:, F]
    return x, atte
